# revision 10
# baseline (speedup 1.0000x reference)
# Trainium2 Bass kernel for BloomStageLoss:
#   loss = mean(label-smoothing CE) + 0.1 * mean(transition penalty)
# over inputs [B, 5] f32, targets [B] int.  B = 4194304, 8 NeuronCores,
# pure data-parallel over the batch; scalar reductions finished on host.
#
# Math (per row i, C=5, s=0.1, smooth=s/(C-1)=0.025):
#   lse_i = ln sum_c exp(x_ic)
#   ce_i  = lse_i - 0.025*rowsum_i - 0.875*x_{i,t_i}
#   pen_i = sum_c P_ic * T[t_i, c],  P = softmax(x),  T[t,c] = phi(|t-c|),
#           phi = [0, .5, 1, 2, 2]
# Exact identity used on-chip (all values exact in f32):
#   m  = 3 - |t - c| ;  r = relu(m) ;  s2 = r + min(r, 1) = 2*(2 - T[t,c])
#   => sum_c P*T = 2*sum_c P - (sum_c P*s2)/2
# One custom 8-stage DVE op computes sum_w P*s2 per class slice (PEN op);
# a second computes sum_w x*(-0.875)*[t==c] (CE op).  sum x goes through
# the TensorEngine (ones-matmul into PSUM).  sum lse via ACT Ln accum.

import os
import sys

sys.path.insert(0, "/opt/trn_rl_repo")

import numpy as np
from contextlib import ExitStack

import concourse.bass as bass
import concourse.bacc as bacc
import concourse.tile as tile
from concourse import mybir
from concourse.bass_utils import run_bass_kernel_spmd

NCORES = 8
C = 5
P = 128
B = 4194304
ROWS = B // NCORES          # 524288 rows per core
W = 1024                    # rows per partition per tile
TILES = ROWS // (P * W)     # 4
SMOOTH_OFF = 0.875          # 1 - SMOOTHING - SMOOTHING/(C-1)
SMOOTH_ALL = 0.025          # SMOOTHING/(C-1)
TPEN = 0.1

_OPS = None


def _register_ops():
    """Define + register the two custom DVE ops (idempotent)."""
    global _OPS
    if _OPS is not None:
        return _OPS
    import concourse.dve_ops as dve_ops
    from concourse.dve_spec import (
        Spec, Src0, Src1, C0, C1, C2, One, relu, minn, lower, AluOp, _has_src1,
    )
    from concourse.dve_uop import DveOpSpec

    def pen_ref(in0, in1, s0, s1, imm2):
        m = np.minimum(s0 - in1, in1 + s1)
        r = np.maximum(m, 0.0)
        s = r + np.minimum(r, 1.0)
        out = (s * in0).astype(np.float32)
        return out, out.reshape(out.shape[0], -1).sum(axis=-1)

    # out = (relu(min(s0-t, t+s1)) + min(relu(.),1)) * in0 ; accum = sum(out)
    _m = minn(C0 - Src1, Src1 + C1)
    _r = relu(_m)
    pen_spec = Spec(body=(_r + minn(_r, One)) * Src0, accum=AluOp.ADD,
                    reference=pen_ref)

    def ce_ref(in0, in1, s0, s1, imm2):
        mask = np.maximum(np.minimum(s0 - in1, in1 + s1), 0.0)
        out = (mask * in0 * imm2).astype(np.float32)
        return out, out.reshape(out.shape[0], -1).sum(axis=-1)

    # out = relu(min(s0-t, t+s1)) * in0 * imm2 ; accum = sum(out)
    ce_spec = Spec(body=relu(minn(C0 - Src1, Src1 + C1)) * Src0 * C2,
                   accum=AluOp.ADD, reference=ce_ref)

    ops = []
    for name, spec in (("PEN_T_ANT", pen_spec), ("CE_SEL_ANT", ce_spec)):
        if name in dve_ops._SUB_OPCODE_FOR_NAME:
            ops.append(next(o for o in dve_ops.OPS if o.name == name))
            continue
        opcode = dve_ops._CUSTOM_DVE_ROW_BASE + len(dve_ops.OPS)
        shas = {}
        for ver in ("v3", "v4"):
            s = DveOpSpec(name=name, opcode=opcode, uops=lower(spec, ver=ver),
                          rd1_en=_has_src1(spec))
            shas[ver] = s.sha(ver)
        op = dve_ops.DveOp(name, spec, subdim=False, uops_sha=shas)
        dve_ops.OPS.append(op)
        dve_ops._SUB_OPCODE_FOR_NAME[name] = opcode
        dve_ops.CUSTOM_DVE_SPECS[name] = spec
        ops.append(op)
    _OPS = tuple(ops)
    return _OPS


def build_nc(rows=ROWS, w=W, ncores=NCORES):
    """Build + compile the single-core program (SPMD across ncores)."""
    pen_op, ce_op = _register_ops()
    tiles = rows // (P * w)
    assert tiles * P * w == rows
    f32 = mybir.dt.float32
    i32 = mybir.dt.int32
    AF = mybir.ActivationFunctionType

    nc = bacc.Bacc("TRN2", target_bir_lowering=False, debug=False,
                   num_devices=ncores)
    x_d = nc.dram_tensor("x", [rows, C], f32, kind="ExternalInput").ap()
    t_d = nc.dram_tensor("t", [rows], i32, kind="ExternalInput").ap()
    lse_d = nc.dram_tensor("lse_acc", [P, tiles], f32, kind="ExternalOutput").ap()
    pen_d = nc.dram_tensor("pen_acc", [P, tiles * C], f32, kind="ExternalOutput").ap()
    ce_d = nc.dram_tensor("ce_acc", [P, tiles * C], f32, kind="ExternalOutput").ap()

    xv = x_d.rearrange("(n p w) c -> n p (w c)", p=P, w=w)
    tv = t_d.rearrange("(n p w) -> n p w", p=P, w=w)
    assert w * C % 512 == 0 or w * C <= 512, "chunk widths must be uniform"
    sxw = min(512, w * C)
    bounds = [(lo, min(lo + 512, w * C)) for lo in range(0, w * C, 512)]
    total_chunks = tiles * len(bounds)
    sx_d = nc.dram_tensor("sumx", [1, sxw], f32, kind="ExternalOutput").ap()

    with tile.TileContext(nc) as tc, ExitStack() as ctx:
        xpool = ctx.enter_context(tc.tile_pool(name="xp", bufs=2))
        tpool = ctx.enter_context(tc.tile_pool(name="tp", bufs=2))
        epool = ctx.enter_context(tc.tile_pool(name="ep", bufs=2))
        ppool = ctx.enter_context(tc.tile_pool(name="pp", bufs=1))
        wpool = ctx.enter_context(tc.tile_pool(name="wp", bufs=2))
        cpool = ctx.enter_context(tc.tile_pool(name="cp", bufs=1))
        spool = ctx.enter_context(tc.tile_pool(name="sp", bufs=1))
        pspool = ctx.enter_context(tc.tile_pool(name="ps", bufs=1, space="PSUM"))

        ones = cpool.tile([P, P], f32)
        nc.vector.memset(ones[:], 1.0)
        lse_acc = spool.tile([P, tiles], f32)
        pen_acc = spool.tile([P, tiles * C], f32)
        ce_acc = spool.tile([P, tiles * C], f32)
        psum_sx = pspool.tile([P, 512], f32)
        sx_sb = cpool.tile([1, sxw], f32)

        chunk = 0
        for n in range(tiles):
            xt = xpool.tile([P, w * C], f32)
            nc.sync.dma_start(xt[:], xv[n])
            tt = tpool.tile([P, w], i32)
            nc.sync.dma_start(tt[:], tv[n])

            tf = tpool.tile([P, w], f32, tag="tf")
            nc.vector.tensor_copy(tf[:], tt[:])

            et = epool.tile([P, w * C], f32)
            nc.scalar.activation(et[:], xt[:], AF.Exp)

            e3 = et[:].rearrange("p (w c) -> p w c", c=C)
            x3 = xt[:].rearrange("p (w c) -> p w c", c=C)

            a = wpool.tile([P, w], f32, tag="tmp")
            b = wpool.tile([P, w], f32, tag="tmp")
            s = wpool.tile([P, w], f32, tag="s")
            nc.vector.tensor_add(a[:], e3[:, :, 0], e3[:, :, 1])
            nc.vector.tensor_add(b[:], e3[:, :, 2], e3[:, :, 3])
            nc.vector.tensor_add(a[:], a[:], b[:])
            nc.vector.tensor_add(s[:], a[:], e3[:, :, 4])

            lnj = wpool.tile([P, w], f32, tag="tmp")
            nc.scalar.activation(lnj[:], s[:], AF.Ln,
                                 accum_out=lse_acc[:, n:n + 1])

            r = wpool.tile([P, w], f32, tag="r")
            nc.vector.reciprocal_approx_fast(r[:], s[:])

            pt = ppool.tile([P, w * C], f32)
            p3 = pt[:].rearrange("p (w c) -> p w c", c=C)
            rb = r[:].unsqueeze(2).broadcast_to([P, w, C])
            nc.vector.tensor_mul(p3, e3, rb)

            scr = wpool.tile([P, w], f32, tag="tmp")
            for cc in range(C):
                nc.vector._custom_dve(
                    pen_op, out=scr[:], in0=p3[:, :, cc], in1=tf[:],
                    s0=3.0 + cc, s1=3.0 - cc,
                    accum_out=pen_acc[:, n * C + cc:n * C + cc + 1])
            for cc in range(C):
                nc.vector._custom_dve(
                    ce_op, out=scr[:], in0=x3[:, :, cc], in1=tf[:],
                    s0=1.0 + cc, s1=1.0 - cc, imm2=-SMOOTH_OFF,
                    accum_out=ce_acc[:, n * C + cc:n * C + cc + 1])

            for lo, hi in bounds:
                nc.tensor.matmul(psum_sx[:, :hi - lo], ones[:],
                                 xt[:, lo:hi],
                                 start=(chunk == 0),
                                 stop=(chunk == total_chunks - 1))
                chunk += 1

        nc.scalar.copy(sx_sb[:], psum_sx[0:1, :sxw])
        nc.sync.dma_start(lse_d, lse_acc[:])
        nc.sync.dma_start(pen_d, pen_acc[:])
        nc.sync.dma_start(ce_d, ce_acc[:])
        nc.sync.dma_start(sx_d, sx_sb[:])

    nc.compile()
    return nc


def combine_host(results, rows_per_core):
    """Fold the per-core accumulator tensors into the scalar loss."""
    tot = 0.0
    n_total = 0
    for res in results:
        lse = np.asarray(res["lse_acc"], np.float64).sum()
        ce_sel = np.asarray(res["ce_acc"], np.float64).sum()   # = -0.875*sum xt
        sumx = np.asarray(res["sumx"], np.float64).sum()
        pen_s2 = np.asarray(res["pen_acc"], np.float64).sum()  # = sum P*s2
        pen = 2.0 * rows_per_core - 0.5 * pen_s2               # = sum_c P*T
        ce = lse + ce_sel - SMOOTH_ALL * sumx
        tot += ce + TPEN * pen
        n_total += rows_per_core
    return np.float32(tot / n_total)


def _ensure_axon_ntff_hook():
    """Provide antenv.axon_hooks if the image lacks it (profiling only)."""
    import importlib
    try:
        importlib.import_module("antenv.axon_hooks")
        return
    except ImportError:
        pass
    import types
    mod = types.ModuleType("antenv.axon_hooks")
    mod._hook = None

    def set_axon_ntff_profile_hook(h):
        mod._hook = h

    def get_axon_ntff_profile_hook():
        if mod._hook is None:
            try:
                from trn_agent_boot.trn_boot import _ntff_profile_via_ctypes
                mod._hook = _ntff_profile_via_ctypes("/opt/axon/libaxon_pjrt.so")
            except Exception:
                mod._hook = None
        return mod._hook

    mod.set_axon_ntff_profile_hook = set_axon_ntff_profile_hook
    mod.get_axon_ntff_profile_hook = get_axon_ntff_profile_hook
    sys.modules["antenv.axon_hooks"] = mod
    try:
        import antenv
        antenv.axon_hooks = mod
    except ImportError:
        pass


_NC_CACHE = None
LAST_RESULTS = None


def kernel(inputs: np.ndarray, targets: np.ndarray) -> np.ndarray:
    global _NC_CACHE, LAST_RESULTS
    x = np.ascontiguousarray(np.asarray(inputs, dtype=np.float32))
    t = np.ascontiguousarray(np.asarray(targets).astype(np.int32))
    assert x.shape == (B, C), x.shape
    assert t.shape == (B,), t.shape

    if _NC_CACHE is None:
        _NC_CACHE = build_nc()
    nc = _NC_CACHE

    in_maps = [
        {"x": x[i * ROWS:(i + 1) * ROWS], "t": t[i * ROWS:(i + 1) * ROWS]}
        for i in range(NCORES)
    ]
    trace = bool(os.environ.get("BASS_TRACE"))
    if trace:
        _ensure_axon_ntff_hook()
    res = run_bass_kernel_spmd(nc, in_maps, list(range(NCORES)), trace=trace)
    LAST_RESULTS = res
    return combine_host(res.results, ROWS)


# revision 12
# speedup vs baseline: 1.1199x; 1.1199x over previous
# Trainium2 Bass kernel for BloomStageLoss:
#   loss = mean(label-smoothing CE) + 0.1 * mean(transition penalty)
# over inputs [B, 5] f32, targets [B] int.  B = 4194304, 8 NeuronCores,
# pure data-parallel over the batch; scalar reductions finished on host.
#
# Math (per row i, C=5, s=0.1, smooth=s/(C-1)=0.025):
#   lse_i = ln sum_c exp(x_ic)
#   ce_i  = lse_i - 0.025*rowsum_i - 0.875*x_{i,t_i}
#   pen_i = sum_c P_ic * T[t_i, c],  P = softmax(x),  T[t,c] = phi(|t-c|),
#           phi = [0, .5, 1, 2, 2]
# Exact identity used on-chip (all values exact in f32):
#   m  = 3 - |t - c| ;  r = relu(m) ;  s2 = r + min(r, 1) = 2*(2 - T[t,c])
#   => sum_c P*T = 2*sum_c P - (sum_c P*s2)/2
# One custom 8-stage DVE op computes sum_w P*s2 per class slice (PEN op);
# a second computes sum_w x*(-0.875)*[t==c] (CE op).  sum x goes through
# the TensorEngine (ones-matmul into PSUM).  sum lse via ACT Ln accum.

import os
import sys

sys.path.insert(0, "/opt/trn_rl_repo")

import numpy as np
from contextlib import ExitStack

import concourse.bass as bass
import concourse.bacc as bacc
import concourse.tile as tile
from concourse import mybir
from concourse.bass_utils import run_bass_kernel_spmd

NCORES = 8
C = 5
P = 128
B = 4194304
ROWS = B // NCORES          # 524288 rows per core
W = 1024                    # rows per partition per tile
TILES = ROWS // (P * W)     # 4
SMOOTH_OFF = 0.875          # 1 - SMOOTHING - SMOOTHING/(C-1)
SMOOTH_ALL = 0.025          # SMOOTHING/(C-1)
TPEN = 0.1

_OPS = None


def _register_ops():
    """Define + register the two custom DVE ops (idempotent)."""
    global _OPS
    if _OPS is not None:
        return _OPS
    import concourse.dve_ops as dve_ops
    from concourse.dve_spec import (
        Spec, Src0, Src1, C0, C1, C2, One, relu, minn, lower, AluOp, _has_src1,
    )
    from concourse.dve_uop import DveOpSpec

    def pen_ref(in0, in1, s0, s1, imm2):
        m = np.minimum(s0 - in1, in1 + s1)
        r = np.maximum(m, 0.0)
        s = r + np.minimum(r, 1.0)
        out = (s * in0).astype(np.float32)
        return out, out.reshape(out.shape[0], -1).sum(axis=-1)

    # out = (relu(min(s0-t, t+s1)) + min(relu(.),1)) * in0 ; accum = sum(out)
    _m = minn(C0 - Src1, Src1 + C1)
    _r = relu(_m)
    pen_spec = Spec(body=(_r + minn(_r, One)) * Src0, accum=AluOp.ADD,
                    reference=pen_ref)

    def ce_ref(in0, in1, s0, s1, imm2):
        mask = np.maximum(np.minimum(s0 - in1, in1 + s1), 0.0)
        out = (mask * in0 * imm2).astype(np.float32)
        return out, out.reshape(out.shape[0], -1).sum(axis=-1)

    # out = relu(min(s0-t, t+s1)) * in0 * imm2 ; accum = sum(out)
    ce_spec = Spec(body=relu(minn(C0 - Src1, Src1 + C1)) * Src0 * C2,
                   accum=AluOp.ADD, reference=ce_ref)

    ops = []
    for name, spec in (("PEN_T_ANT", pen_spec), ("CE_SEL_ANT", ce_spec)):
        if name in dve_ops._SUB_OPCODE_FOR_NAME:
            ops.append(next(o for o in dve_ops.OPS if o.name == name))
            continue
        opcode = dve_ops._CUSTOM_DVE_ROW_BASE + len(dve_ops.OPS)
        shas = {}
        for ver in ("v3", "v4"):
            s = DveOpSpec(name=name, opcode=opcode, uops=lower(spec, ver=ver),
                          rd1_en=_has_src1(spec))
            shas[ver] = s.sha(ver)
        op = dve_ops.DveOp(name, spec, subdim=False, uops_sha=shas)
        dve_ops.OPS.append(op)
        dve_ops._SUB_OPCODE_FOR_NAME[name] = opcode
        dve_ops.CUSTOM_DVE_SPECS[name] = spec
        ops.append(op)
    _OPS = tuple(ops)
    return _OPS


def build_nc(rows=ROWS, w=W, ncores=NCORES):
    """Build + compile the single-core program (SPMD across ncores)."""
    pen_op, ce_op = _register_ops()
    tiles = rows // (P * w)
    assert tiles * P * w == rows
    f32 = mybir.dt.float32
    i32 = mybir.dt.int32
    AF = mybir.ActivationFunctionType

    nc = bacc.Bacc("TRN2", target_bir_lowering=False, debug=False,
                   num_devices=ncores)
    x_d = nc.dram_tensor("x", [rows, C], f32, kind="ExternalInput").ap()
    t_d = nc.dram_tensor("t", [rows], i32, kind="ExternalInput").ap()
    lse_d = nc.dram_tensor("lse_acc", [P, tiles], f32, kind="ExternalOutput").ap()
    pen_d = nc.dram_tensor("pen_acc", [P, tiles * C], f32, kind="ExternalOutput").ap()
    ce_d = nc.dram_tensor("ce_acc", [P, tiles * C], f32, kind="ExternalOutput").ap()

    xv = x_d.rearrange("(n p w) c -> n p (w c)", p=P, w=w)
    tv = t_d.rearrange("(n p w) -> n p w", p=P, w=w)
    assert w * C % 512 == 0 or w * C <= 512, "chunk widths must be uniform"
    sxw = min(512, w * C)
    bounds = [(lo, min(lo + 512, w * C)) for lo in range(0, w * C, 512)]
    total_chunks = tiles * len(bounds)
    sx_d = nc.dram_tensor("sumx", [1, sxw], f32, kind="ExternalOutput").ap()

    with tile.TileContext(nc) as tc, ExitStack() as ctx:
        xpool = ctx.enter_context(tc.tile_pool(name="xp", bufs=2))
        tpool = ctx.enter_context(tc.tile_pool(name="tp", bufs=2))
        epool = ctx.enter_context(tc.tile_pool(name="ep", bufs=2))
        ppool = ctx.enter_context(tc.tile_pool(name="pp", bufs=1))
        wpool = ctx.enter_context(tc.tile_pool(name="wp", bufs=2))
        cpool = ctx.enter_context(tc.tile_pool(name="cp", bufs=1))
        spool = ctx.enter_context(tc.tile_pool(name="sp", bufs=1))
        pspool = ctx.enter_context(tc.tile_pool(name="ps", bufs=1, space="PSUM"))

        ones = cpool.tile([P, P], f32)
        nc.vector.memset(ones[:], 1.0)
        lse_acc = spool.tile([P, tiles], f32)
        pen_acc = spool.tile([P, tiles * C], f32)
        ce_acc = spool.tile([P, tiles * C], f32)
        psum_sx = pspool.tile([P, 512], f32)
        sx_sb = cpool.tile([1, sxw], f32)

        s_list = [spool.tile([P, w], f32, name=f"s{n}", tag=f"s{n}")
                  for n in range(tiles)]

        chunk = 0
        for n in range(tiles):
            xt = xpool.tile([P, w * C], f32)
            nc.sync.dma_start(xt[:], xv[n])
            tt = tpool.tile([P, w], i32)
            nc.sync.dma_start(tt[:], tv[n])

            tf = tpool.tile([P, w], f32, tag="tf")
            nc.vector.tensor_copy(tf[:], tt[:])

            x3 = xt[:].rearrange("p (w c) -> p w c", c=C)

            # exp, de-interleaved: et is c-blocked [E0|E1|E2|E3|E4], dense
            et = epool.tile([P, w * C], f32)
            for cc in range(C):
                nc.scalar.activation(et[:, cc * w:(cc + 1) * w],
                                     x3[:, :, cc], AF.Exp)

            a = wpool.tile([P, w], f32, tag="tmp")
            b = wpool.tile([P, w], f32, tag="tmp")
            s = s_list[n]
            nc.vector.tensor_add(a[:], et[:, 0:w], et[:, w:2 * w])
            nc.vector.tensor_add(b[:], et[:, 2 * w:3 * w], et[:, 3 * w:4 * w])
            nc.vector.tensor_add(a[:], a[:], b[:])
            nc.vector.tensor_add(s[:], a[:], et[:, 4 * w:5 * w])

            r = wpool.tile([P, w], f32, tag="r")
            nc.vector.reciprocal_approx_fast(r[:], s[:])

            # P = E * r (row-broadcast over the c-blocked layout), dense
            pt = ppool.tile([P, w * C], f32)
            p3 = pt[:].rearrange("p (c w) -> p c w", c=C)
            e3b = et[:].rearrange("p (c w) -> p c w", c=C)
            rb = r[:].unsqueeze(1).broadcast_to([P, C, w])
            nc.vector.tensor_mul(p3, e3b, rb)

            scr = wpool.tile([P, w], f32, tag="tmp")
            for cc in range(C):
                nc.vector._custom_dve(
                    pen_op, out=scr[:], in0=pt[:, cc * w:(cc + 1) * w],
                    in1=tf[:], s0=3.0 + cc, s1=3.0 - cc,
                    accum_out=pen_acc[:, n * C + cc:n * C + cc + 1])
            for cc in range(C):
                nc.vector._custom_dve(
                    ce_op, out=scr[:], in0=x3[:, :, cc], in1=tf[:],
                    s0=1.0 + cc, s1=1.0 - cc, imm2=-SMOOTH_OFF,
                    accum_out=ce_acc[:, n * C + cc:n * C + cc + 1])

            for lo, hi in bounds:
                nc.tensor.matmul(psum_sx[:, :hi - lo], ones[:],
                                 xt[:, lo:hi],
                                 start=(chunk == 0),
                                 stop=(chunk == total_chunks - 1))
                chunk += 1

        # all Ln at the end: one table-set switch instead of per-tile thrash
        for n in range(tiles):
            lnj = wpool.tile([P, w], f32, tag="tmp")
            nc.scalar.activation(lnj[:], s_list[n][:], AF.Ln,
                                 accum_out=lse_acc[:, n:n + 1])

        nc.scalar.copy(sx_sb[:], psum_sx[0:1, :sxw])
        nc.sync.dma_start(lse_d, lse_acc[:])
        nc.sync.dma_start(pen_d, pen_acc[:])
        nc.sync.dma_start(ce_d, ce_acc[:])
        nc.sync.dma_start(sx_d, sx_sb[:])

    nc.compile()
    return nc


def combine_host(results, rows_per_core):
    """Fold the per-core accumulator tensors into the scalar loss."""
    tot = 0.0
    n_total = 0
    for res in results:
        lse = np.asarray(res["lse_acc"], np.float64).sum()
        ce_sel = np.asarray(res["ce_acc"], np.float64).sum()   # = -0.875*sum xt
        sumx = np.asarray(res["sumx"], np.float64).sum()
        pen_s2 = np.asarray(res["pen_acc"], np.float64).sum()  # = sum P*s2
        pen = 2.0 * rows_per_core - 0.5 * pen_s2               # = sum_c P*T
        ce = lse + ce_sel - SMOOTH_ALL * sumx
        tot += ce + TPEN * pen
        n_total += rows_per_core
    return np.float32(tot / n_total)


def _ensure_axon_ntff_hook():
    """Provide antenv.axon_hooks if the image lacks it (profiling only)."""
    import importlib
    try:
        importlib.import_module("antenv.axon_hooks")
        return
    except ImportError:
        pass
    import types
    mod = types.ModuleType("antenv.axon_hooks")
    mod._hook = None

    def set_axon_ntff_profile_hook(h):
        mod._hook = h

    def get_axon_ntff_profile_hook():
        if mod._hook is None:
            try:
                from trn_agent_boot.trn_boot import _ntff_profile_via_ctypes
                mod._hook = _ntff_profile_via_ctypes("/opt/axon/libaxon_pjrt.so")
            except Exception:
                mod._hook = None
        return mod._hook

    mod.set_axon_ntff_profile_hook = set_axon_ntff_profile_hook
    mod.get_axon_ntff_profile_hook = get_axon_ntff_profile_hook
    sys.modules["antenv.axon_hooks"] = mod
    try:
        import antenv
        antenv.axon_hooks = mod
    except ImportError:
        pass


_NC_CACHE = None
LAST_RESULTS = None


def kernel(inputs: np.ndarray, targets: np.ndarray) -> np.ndarray:
    global _NC_CACHE, LAST_RESULTS
    x = np.ascontiguousarray(np.asarray(inputs, dtype=np.float32))
    t = np.ascontiguousarray(np.asarray(targets).astype(np.int32))
    assert x.shape == (B, C), x.shape
    assert t.shape == (B,), t.shape

    if _NC_CACHE is None:
        _NC_CACHE = build_nc()
    nc = _NC_CACHE

    in_maps = [
        {"x": x[i * ROWS:(i + 1) * ROWS], "t": t[i * ROWS:(i + 1) * ROWS]}
        for i in range(NCORES)
    ]
    trace = bool(os.environ.get("BASS_TRACE"))
    if trace:
        _ensure_axon_ntff_hook()
    res = run_bass_kernel_spmd(nc, in_maps, list(range(NCORES)), trace=trace)
    LAST_RESULTS = res
    return combine_host(res.results, ROWS)
